# revision 1
# baseline (speedup 1.0000x reference)
"""CCPL contrastive-loss kernel for Trainium2 (8 NeuronCores).

Strategy: the loss only touches 256 sampled 3x3 neighborhoods of
feat_q/feat_k (~4.7 MB of each 512 MiB tensor), so the kernel never
streams the full tensors.  Work is data-parallel over the batch dim:
core b receives feat_q[b] / feat_k[b] (64 MiB each staged to HBM) and a
program with the 256 sample windows baked in as static strided DMAs
(sample_ids are host-known at build time, identical for every core, so
the program is SPMD-clean).  Each core gathers [64c, 256s, 9] blocks for
q and k, normalizes over the channel dim, and emits one partial
sum(|q_hat - k_hat|); the host sums the 8 partials and divides by the
element count.
"""

import os
import sys
from contextlib import ExitStack

import numpy as np

sys.path.insert(0, "/opt/trn_rl_repo")

import concourse.bass as bass
import concourse.tile as tile
from concourse import mybir
from concourse.bass_utils import run_bass_kernel_spmd


def _install_ntff_hook():
    """Provide antenv.axon_hooks when the agent image lacks it.

    concourse's axon trace path imports antenv.axon_hooks to fetch the
    NTFF profile hook; this image's antenv has no such submodule.  The
    hook implementation ships in trn_agent_boot.trn_boot, so wire it up
    against the axon PJRT .so directly.
    """
    try:
        from antenv.axon_hooks import get_axon_ntff_profile_hook  # noqa: F401

        return
    except ImportError:
        pass
    import types

    hook = None
    try:
        from trn_agent_boot.trn_boot import _ntff_profile_via_ctypes

        so = "/opt/axon/libaxon_pjrt.so"
        if os.path.exists(so):
            hook = _ntff_profile_via_ctypes(so)
    except Exception:
        hook = None
    mod = types.ModuleType("antenv.axon_hooks")
    _state = {"hook": hook}
    mod.get_axon_ntff_profile_hook = lambda: _state["hook"]
    mod.set_axon_ntff_profile_hook = lambda h: _state.update(hook=h)
    import antenv

    sys.modules["antenv.axon_hooks"] = mod
    antenv.axon_hooks = mod


_install_ntff_hook()

B, C, H, W = 8, 64, 512, 512
NUM_S = 256
EPS = 1e-7
NCOL = NUM_S * 9  # 2304 columns: (sample, 3x3 window) with center at j=4
CHUNK = 384  # matmul moving-free <= 512; 6 even chunks
NCHUNK = NCOL // CHUNK
N_CORES = 8

_cache: dict = {}
LAST_RESULTS = None  # BassKernelResults of the most recent run (for test.py)


def _split_multi_waits(nc):
    """Walrus build here embeds at most ONE sync wait per instruction.

    Tile emits instructions (notably the kernel-tail Drain) carrying many
    sem waits.  Hoist all but the last wait of any such instruction onto
    single-wait NOPs inserted immediately before it on the same queue —
    the queue stalls on each NOP in turn, preserving semantics.
    """
    from concourse import mybir as _mybir

    for f in nc.m.functions:
        for blk in f.blocks:
            insts = blk.instructions
            i = 0
            while i < len(insts):
                inst = insts[i]
                si = inst.sync_info
                if si is not None and si.on_wait and len(si.on_wait) > 1:
                    waits = list(si.on_wait)
                    si.on_wait = waits[-1:]
                    for j, w in enumerate(waits[:-1]):
                        nop = _mybir.InstNoOp(
                            name=nc.get_next_instruction_name(),
                            ins=[],
                            outs=[],
                            engine=inst.engine,
                            sync_info=_mybir.SyncInfo(on_wait=[w], on_update=[]),
                        )
                        insts.insert(i + j, nop)
                    i += len(waits) - 1
                i += 1


def _build(ids):
    f32 = mybir.dt.float32
    P = 2 * C  # q on partitions 0-63, k on 64-127
    nc = bass.Bass()
    # q and k stacked: the (tensor, channel) dims merge into one uniform
    # 128-row stride, so a single DMA per sample feeds all 16 SDMA ports.
    fqk = nc.dram_tensor("fqk", [P, H, W], f32, kind="ExternalInput")
    # [I64; -I64] so (q_hat - k_hat) falls out of one K=128 matmul
    wdiff = nc.dram_tensor("wdiff", [P, C], f32, kind="ExternalInput")
    out = nc.dram_tensor("out", [1, 1], f32, kind="ExternalOutput")

    with tile.TileContext(nc) as tc, ExitStack() as ctx:
        sb = ctx.enter_context(tc.tile_pool(name="sb", bufs=1))
        work = ctx.enter_context(tc.tile_pool(name="work", bufs=3))
        pn = ctx.enter_context(tc.tile_pool(name="pn", bufs=1, space="PSUM"))
        pbc = ctx.enter_context(tc.tile_pool(name="pbc", bufs=2, space="PSUM"))
        pd = ctx.enter_context(tc.tile_pool(name="pd", bufs=2, space="PSUM"))
        pf = ctx.enter_context(tc.tile_pool(name="pf", bufs=1, space="PSUM"))

        ones = sb.tile([P, 1], f32)
        nc.vector.memset(ones[:], 1.0)
        ones_row = sb.tile([1, C], f32)
        nc.vector.memset(ones_row[:], 1.0)
        wd = sb.tile([P, C], f32)
        nc.sync.dma_start(out=wd[:], in_=wdiff[:])
        # PE warmup so later matmuls don't pay a fresh DVE-clock wait.
        warm = pf.tile([1, 1], f32, tag="warm")
        nc.tensor.matmul(
            out=warm[:], lhsT=ones[:], rhs=ones[:], start=True, stop=True
        )

        qkraw = sb.tile([P, NUM_S, 9], f32)
        # Gather 3x3 windows: ONE strided DMA per sample covering q and k
        # (12B contiguous runs x 3 rows x 128 stacked channels).  The
        # bottleneck is descriptor generation (~4 ns/descriptor per ring),
        # so spread samples over all three generators: SP and ACT HWDGE
        # rings plus the gpsimd SWDGE ring (a bit slower per descriptor).
        qeng = [
            nc.sync, nc.scalar, nc.gpsimd, nc.sync,
            nc.scalar, nc.sync, nc.scalar, nc.gpsimd,
        ]
        for s, (h, w) in enumerate(ids):
            qeng[s % 8].dma_start(
                out=qkraw[:, s, :], in_=fqk[:, h : h + 3, w : w + 3]
            )

        # Process samples in groups so compute streams behind the gathers.
        GS = 32  # samples per group
        GC = GS * 9  # 288 columns (matmul moving-free <= 512)
        NG = NUM_S // GS
        d = sb.tile([P, NUM_S, 9], f32)
        d2 = sb.tile([P, NUM_S, 9], f32)
        df_ = d[:].rearrange("p s n -> p (s n)")
        d2f = d2[:].rearrange("p s n -> p (s n)")
        # q norms in cols [0, NCOL), k norms in cols [NCOL, 2*NCOL): engine
        # writes must stay at partition base 0
        norm = sb.tile([1, 2 * NCOL], f32)
        rinv = sb.tile([1, 2 * NCOL], f32)
        acc = sb.tile([C, NG], f32)

        for g in range(NG):
            ss = slice(g * GS, (g + 1) * GS)
            sl = slice(g * GC, (g + 1) * GC)
            slk = slice(NCOL + g * GC, NCOL + (g + 1) * GC)
            # d = window - center (center column j=4 becomes exactly 0)
            nc.vector.tensor_tensor(
                out=d[:, ss, :],
                in0=qkraw[:, ss, :],
                in1=qkraw[:, ss, 4:5].to_broadcast([P, GS, 9]),
                op=mybir.AluOpType.subtract,
            )
            nc.scalar.square(out=d2[:, ss, :], in_=d[:, ss, :])
            # norm2[col] = sum_c d2[c, col], q and k halves separately
            n2q = pn.tile([1, GC], f32, tag="n2q")
            n2k = pn.tile([1, GC], f32, tag="n2k")
            nc.tensor.matmul(
                out=n2q[:], lhsT=ones[0:C, :], rhs=d2f[0:C, sl],
                start=True, stop=True,
            )
            nc.tensor.matmul(
                out=n2k[:], lhsT=ones[C:P, :], rhs=d2f[C:P, sl],
                start=True, stop=True,
            )
            nc.scalar.sqrt(out=norm[:, sl], in_=n2q[:])
            nc.scalar.sqrt(out=norm[:, slk], in_=n2k[:])
            # rinv = 1/(sqrt(norm2)+eps); center cols give d*(1/eps) = 0
            nc.vector.tensor_scalar_add(
                out=norm[:, sl], in0=norm[:, sl], scalar1=EPS
            )
            nc.vector.tensor_scalar_add(
                out=norm[:, slk], in0=norm[:, slk], scalar1=EPS
            )
            nc.vector.reciprocal(out=rinv[:, sl], in_=norm[:, sl])
            nc.vector.reciprocal(out=rinv[:, slk], in_=norm[:, slk])
            # two K=1 matmuls broadcast rinv_q/rinv_k onto partition
            # quadrants 0 and 64 of one PSUM tile
            bc = pbc.tile([P, GC], f32)
            nc.tensor.matmul(
                out=bc[0:C, :], lhsT=ones_row[:], rhs=rinv[:, sl],
                start=True, stop=True,
            )
            nc.tensor.matmul(
                out=bc[C:P, :], lhsT=ones_row[:], rhs=rinv[:, slk],
                start=True, stop=True,
            )
            qkh = work.tile([P, GC], f32, tag="qkh")
            nc.vector.tensor_tensor(
                out=qkh[:], in0=df_[:, sl], in1=bc[:], op=mybir.AluOpType.mult
            )
            # q_hat - k_hat across the partition halves via [I; -I] matmul
            dif = pd.tile([C, GC], f32, tag="dif")
            nc.tensor.matmul(
                out=dif[:], lhsT=wd[:], rhs=qkh[:], start=True, stop=True
            )
            nc.vector.tensor_reduce(
                out=acc[:, g : g + 1],
                in_=dif[:],
                axis=mybir.AxisListType.X,
                op=mybir.AluOpType.add,
                apply_absolute_value=True,
            )

        accs = sb.tile([C, 1], f32)
        nc.vector.tensor_reduce(
            out=accs[:], in_=acc[:], axis=mybir.AxisListType.X, op=mybir.AluOpType.add
        )
        pfin = pf.tile([1, 1], f32, tag="fin")
        nc.tensor.matmul(
            out=pfin[:], lhsT=accs[:], rhs=ones[0:C, :], start=True, stop=True
        )
        res = sb.tile([1, 1], f32)
        nc.scalar.copy(out=res[:], in_=pfin[:])
        nc.gpsimd.dma_start(out=out[:], in_=res[:])

    _split_multi_waits(nc)
    return nc


def kernel(feat_q, feat_k, sample_ids, *, trace=False, trace_cores=None):
    global LAST_RESULTS
    feat_q = np.ascontiguousarray(np.asarray(feat_q), dtype=np.float32)
    feat_k = np.ascontiguousarray(np.asarray(feat_k), dtype=np.float32)
    ids = np.asarray(sample_ids)
    ids_key = tuple(map(tuple, ids.astype(np.int64).tolist()))
    if ids_key not in _cache:
        _cache[ids_key] = _build(ids_key)
    nc = _cache[ids_key]

    eye = np.eye(C, dtype=np.float32)
    wd = np.concatenate([eye, -eye], axis=0)  # [128, 64]
    in_maps = [
        {
            "fqk": np.concatenate([feat_q[b], feat_k[b]], axis=0),
            "wdiff": wd,
        }
        for b in range(N_CORES)
    ]
    results = run_bass_kernel_spmd(
        nc,
        in_maps,
        core_ids=list(range(N_CORES)),
        trace=trace,
        trace_cores=trace_cores,
    )
    LAST_RESULTS = results
    total = np.float64(0.0)
    for r in results.results:
        total += np.float64(r["out"][0, 0])
    loss = total / (B * C * 8 * NUM_S)
    return np.asarray(loss, dtype=np.float32)



# revision 3
# speedup vs baseline: 5.6493x; 5.6493x over previous
"""CCPL contrastive-loss kernel for Trainium2 (8 NeuronCores).

Strategy: the loss only touches 256 sampled 3x3 neighborhoods of
feat_q/feat_k, so the kernel never streams the full tensors.  Work is
data-parallel over the batch dim: core b gets batch b's q and k stacked
channels-LAST as one [H*W, 2C] array in HBM.  In that layout a window
row (3 cols x 128 channels) is one contiguous 1536B run, and landing
each sample on its own SBUF partition makes it a single DMA descriptor.
The whole gather is then ONE gpsimd indirect DMA driven by a [128, 6]
int32 row-index tensor (6 window rows per partition: 2 samples x 3
rows), instead of hundreds of strided dma_starts paying the ~630ns
fixed HWDGE cost each.

Compute is fully 128-lane: samples live on partitions, (sample-half,
position, channel) on the free dim.  d = window - center; norm2 via
ACT square + DVE grouped reduce over the 64-channel runs; rinv =
1/(sqrt+eps); |q_hat - k_hat| abs-summed per partition; one final
matmul folds partitions; host sums the 8 per-core partials and divides
by the element count.
"""

import os
import sys
from contextlib import ExitStack

import numpy as np

sys.path.insert(0, "/opt/trn_rl_repo")

import concourse.bass as bass
import concourse.tile as tile
from concourse import mybir
from concourse.bass_utils import run_bass_kernel_spmd


def _install_ntff_hook():
    """Provide antenv.axon_hooks when the agent image lacks it.

    concourse's axon trace path imports antenv.axon_hooks to fetch the
    NTFF profile hook; this image's antenv has no such submodule.  The
    hook implementation ships in trn_agent_boot.trn_boot, so wire it up
    against the axon PJRT .so directly.
    """
    try:
        from antenv.axon_hooks import get_axon_ntff_profile_hook  # noqa: F401

        return
    except ImportError:
        pass
    import types

    hook = None
    try:
        from trn_agent_boot.trn_boot import _ntff_profile_via_ctypes

        so = "/opt/axon/libaxon_pjrt.so"
        if os.path.exists(so):
            hook = _ntff_profile_via_ctypes(so)
    except Exception:
        hook = None
    mod = types.ModuleType("antenv.axon_hooks")
    _state = {"hook": hook}
    mod.get_axon_ntff_profile_hook = lambda: _state["hook"]
    mod.set_axon_ntff_profile_hook = lambda h: _state.update(hook=h)
    import antenv

    sys.modules["antenv.axon_hooks"] = mod
    antenv.axon_hooks = mod


_install_ntff_hook()

B, C, H, W = 8, 64, 512, 512
NUM_S = 256
EPS = 1e-7
P = 128  # samples per partition-layer; 2C stacked channels
NSHI = NUM_S // P  # 2 sample layers per partition
NROW = 3 * NSHI  # window rows gathered per partition
N_CORES = 8

_nc_cache = None
LAST_RESULTS = None  # BassKernelResults of the most recent run (for test.py)


def _split_multi_waits(nc):
    """Walrus build here embeds at most ONE sync wait per instruction.

    Tile emits instructions (notably the kernel-tail Drain) carrying many
    sem waits.  Hoist all but the last wait of any such instruction onto
    single-wait NOPs inserted immediately before it on the same queue —
    the queue stalls on each NOP in turn, preserving semantics.
    """
    from concourse import mybir as _mybir

    for f in nc.m.functions:
        for blk in f.blocks:
            insts = blk.instructions
            i = 0
            while i < len(insts):
                inst = insts[i]
                si = inst.sync_info
                if si is not None and si.on_wait and len(si.on_wait) > 1:
                    waits = list(si.on_wait)
                    si.on_wait = waits[-1:]
                    for j, w in enumerate(waits[:-1]):
                        nop = _mybir.InstNoOp(
                            name=nc.get_next_instruction_name(),
                            ins=[],
                            outs=[],
                            engine=inst.engine,
                            sync_info=_mybir.SyncInfo(on_wait=[w], on_update=[]),
                        )
                        insts.insert(i + j, nop)
                    i += len(waits) - 1
                i += 1


def _build():
    f32 = mybir.dt.float32
    i32 = mybir.dt.int32
    nc = bass.Bass()
    # channels-last: flat (h*W + w) rows of 2C channels (q in 0..C, k in C..2C)
    fqkT = nc.dram_tensor("fqkT", [H * W, 2 * C], f32, kind="ExternalInput")
    # per (partition, shi*3 + r): DRAM row index (h+r)*W + w of sample shi*128+p
    idx = nc.dram_tensor("idx", [P, NROW], i32, kind="ExternalInput")
    out = nc.dram_tensor("out", [1, 1], f32, kind="ExternalOutput")

    with tile.TileContext(nc) as tc, ExitStack() as ctx:
        sb = ctx.enter_context(tc.tile_pool(name="sb", bufs=1))
        pf = ctx.enter_context(tc.tile_pool(name="pf", bufs=1, space="PSUM"))

        idx_sb = sb.tile([P, NROW], i32)
        nc.sync.dma_start(out=idx_sb[:], in_=idx[:])
        ones = sb.tile([P, 1], f32)
        nc.vector.memset(ones[:], 1.0)
        # PE warmup so the final matmul doesn't pay a fresh clock wait.
        warm = pf.tile([1, 1], f32, tag="warm")
        nc.tensor.matmul(
            out=warm[:], lhsT=ones[:], rhs=ones[:], start=True, stop=True
        )

        # One indirect gather: per index, 384 consecutive floats from fqkT
        # (= 3 w-positions x 128 channels) land contiguously in one
        # partition.  Free-dim layout per partition: (shi, pos, ch).
        qk = sb.tile([P, NSHI, 9, 2 * C], f32)
        nc.gpsimd.indirect_dma_start(
            out=qk[:].rearrange("p s n c -> p (s n c)"),
            out_offset=None,
            in_=fqkT[:],
            in_offset=bass.IndirectOffsetOnAxis(ap=idx_sb[:], axis=0),
        )

        d = sb.tile([P, NSHI, 9, 2 * C], f32)
        d2 = sb.tile([P, NSHI, 9, 2 * C], f32)
        # 18 groups of (t=q|k, 64ch) per partition
        n2 = sb.tile([P, NSHI * 9, 2], f32)
        rinv = sb.tile([P, NSHI * 9, 2, 1], f32)
        dif = sb.tile([P, NSHI * 9, C], f32)
        acc = sb.tile([P, 1], f32)

        # d = window - center (center column pos=4 becomes exactly 0)
        nc.vector.tensor_tensor(
            out=d[:],
            in0=qk[:],
            in1=qk[:, :, 4:5, :].to_broadcast([P, NSHI, 9, 2 * C]),
            op=mybir.AluOpType.subtract,
        )
        nc.scalar.square(out=d2[:], in_=d[:])
        # norm2 over each 64-channel run, q and k separately
        nc.vector.tensor_reduce(
            out=n2[:],
            in_=d2[:].rearrange("p s n (t c) -> p (s n) t c", t=2),
            axis=mybir.AxisListType.X,
            op=mybir.AluOpType.add,
        )
        nc.scalar.sqrt(out=n2[:], in_=n2[:])
        nc.vector.tensor_scalar_add(out=n2[:], in0=n2[:], scalar1=EPS)
        nc.vector.reciprocal(out=rinv[:], in_=n2[:])
        # qhat/khat = d * rinv (broadcast rinv over the 64-channel runs);
        # center columns give d * (1/eps) = 0
        qh = d2  # reuse; d2 no longer needed
        nc.vector.tensor_tensor(
            out=qh[:].rearrange("p s n (t c) -> p (s n) t c", t=2),
            in0=d[:].rearrange("p s n (t c) -> p (s n) t c", t=2),
            in1=rinv[:].to_broadcast([P, NSHI * 9, 2, C]),
            op=mybir.AluOpType.mult,
        )
        qhv = qh[:].rearrange("p s n (t c) -> p (s n) t c", t=2)
        nc.vector.tensor_tensor(
            out=dif[:],
            in0=qhv[:, :, 0, :],
            in1=qhv[:, :, 1, :],
            op=mybir.AluOpType.subtract,
        )
        nc.vector.tensor_reduce(
            out=acc[:],
            in_=dif[:],
            axis=mybir.AxisListType.XY,
            op=mybir.AluOpType.add,
            apply_absolute_value=True,
        )
        pfin = pf.tile([1, 1], f32, tag="fin")
        nc.tensor.matmul(
            out=pfin[:], lhsT=acc[:], rhs=ones[:], start=True, stop=True
        )
        res = sb.tile([1, 1], f32)
        nc.scalar.copy(out=res[:], in_=pfin[:])
        nc.sync.dma_start(out=out[:], in_=res[:])

    _split_multi_waits(nc)
    return nc


def kernel(feat_q, feat_k, sample_ids, *, trace=False, trace_cores=None):
    global LAST_RESULTS, _nc_cache
    feat_q = np.asarray(feat_q, dtype=np.float32)
    feat_k = np.asarray(feat_k, dtype=np.float32)
    ids = np.asarray(sample_ids).astype(np.int64)
    if _nc_cache is None:
        _nc_cache = _build()
    nc = _nc_cache

    # idx[p, shi*3 + r] = (h + r)*W + w for sample s = shi*128 + p
    hw = ids[:, 0] * W + ids[:, 1]  # [256]
    rows = hw[:, None] + np.arange(3, dtype=np.int64)[None, :] * W  # [256, 3]
    idx_np = np.ascontiguousarray(
        rows.reshape(NSHI, P, 3).transpose(1, 0, 2).reshape(P, NROW)
    ).astype(np.int32)

    in_maps = []
    for b in range(N_CORES):
        x = np.concatenate([feat_q[b], feat_k[b]], axis=0)  # [128, H, W]
        fqkT = np.ascontiguousarray(x.transpose(1, 2, 0)).reshape(H * W, 2 * C)
        in_maps.append({"fqkT": fqkT, "idx": idx_np})

    results = run_bass_kernel_spmd(
        nc,
        in_maps,
        core_ids=list(range(N_CORES)),
        trace=trace,
        trace_cores=trace_cores,
    )
    LAST_RESULTS = results
    total = np.float64(0.0)
    for r in results.results:
        total += np.float64(r["out"][0, 0])
    loss = total / (B * C * 8 * NUM_S)
    return np.asarray(loss, dtype=np.float32)


# revision 7
# speedup vs baseline: 5.8992x; 1.0442x over previous
"""CCPL contrastive-loss kernel for Trainium2 (8 NeuronCores).

Strategy: the loss only touches 256 sampled 3x3 neighborhoods of
feat_q/feat_k, so the kernel never streams the full tensors.  Work is
data-parallel over the batch dim: core b gets batch b's q and k stacked
channels-LAST as one [H*W, 2C] array in HBM.  In that layout a window
row (3 cols x 128 channels) is one contiguous 1536B run, and landing
each sample on its own SBUF partition makes it a single DMA descriptor.
The whole gather is TWO gpsimd indirect DMAs (one per 128-sample
layer, pipelined against compute) driven by a [128, 6] int32 row-index
tensor, instead of hundreds of strided dma_starts paying the ~630ns
fixed HWDGE cost each.

Compute is fully 128-lane: samples live on partitions, (position,
channel) on the free dim.  d = window - center; norm2 via ACT square +
DVE grouped reduce over the 64-channel runs; nrm = sqrt(n2 + 1e-14)
(== sqrt(n2) + 1e-7 at n2 == 0, matching the reference eps exactly
where it matters); rinv = 1/nrm; qhat = d * rinv; q-k difference on
gpsimd; |dif| abs-summed per partition.  The per-partition partials
[128, 2] go back to DRAM and the host does the final tiny sum.
"""

import os
import sys
from contextlib import ExitStack

import numpy as np

sys.path.insert(0, "/opt/trn_rl_repo")

import concourse.bass as bass
import concourse.tile as tile
from concourse import mybir
from concourse.bass_utils import run_bass_kernel_spmd


def _install_ntff_hook():
    """Provide antenv.axon_hooks when the agent image lacks it.

    concourse's axon trace path imports antenv.axon_hooks to fetch the
    NTFF profile hook; this image's antenv has no such submodule.  The
    hook implementation ships in trn_agent_boot.trn_boot, so wire it up
    against the axon PJRT .so directly.
    """
    try:
        from antenv.axon_hooks import get_axon_ntff_profile_hook  # noqa: F401

        return
    except ImportError:
        pass
    import types

    hook = None
    try:
        from trn_agent_boot.trn_boot import _ntff_profile_via_ctypes

        so = "/opt/axon/libaxon_pjrt.so"
        if os.path.exists(so):
            hook = _ntff_profile_via_ctypes(so)
    except Exception:
        hook = None
    mod = types.ModuleType("antenv.axon_hooks")
    _state = {"hook": hook}
    mod.get_axon_ntff_profile_hook = lambda: _state["hook"]
    mod.set_axon_ntff_profile_hook = lambda h: _state.update(hook=h)
    import antenv

    sys.modules["antenv.axon_hooks"] = mod
    antenv.axon_hooks = mod


_install_ntff_hook()

B, C, H, W = 8, 64, 512, 512
NUM_S = 256
DELTA = 1e-14  # sqrt(n2 + DELTA): equals sqrt(n2)+1e-7 at n2==0 (center cols)
P = 128  # samples per partition-layer; 2C stacked channels
NSHI = NUM_S // P  # 2 sample layers per partition
NROW = 3 * NSHI  # window rows gathered per partition
N_CORES = 8

_nc_cache = None
LAST_RESULTS = None  # BassKernelResults of the most recent run (for test.py)


def _split_multi_waits(nc):
    """Walrus build here embeds at most ONE sync wait per instruction.

    Tile emits instructions (notably the kernel-tail Drain) carrying many
    sem waits.  Hoist all but the last wait of any such instruction onto
    single-wait NOPs inserted immediately before it on the same queue —
    the queue stalls on each NOP in turn, preserving semantics.
    """
    from concourse import mybir as _mybir

    for f in nc.m.functions:
        for blk in f.blocks:
            insts = blk.instructions
            i = 0
            while i < len(insts):
                inst = insts[i]
                si = inst.sync_info
                if si is not None and si.on_wait and len(si.on_wait) > 1:
                    waits = list(si.on_wait)
                    si.on_wait = waits[-1:]
                    for j, w in enumerate(waits[:-1]):
                        nop = _mybir.InstNoOp(
                            name=nc.get_next_instruction_name(),
                            ins=[],
                            outs=[],
                            engine=inst.engine,
                            sync_info=_mybir.SyncInfo(on_wait=[w], on_update=[]),
                        )
                        insts.insert(i + j, nop)
                    i += len(waits) - 1
                i += 1


def _build():
    f32 = mybir.dt.float32
    i32 = mybir.dt.int32
    nc = bass.Bass()
    # channels-last: flat (h*W + w) rows of 2C channels (q in 0..C, k in C..2C)
    fqkT = nc.dram_tensor("fqkT", [H * W, 2 * C], f32, kind="ExternalInput")
    # per (partition, shi*3 + r): DRAM row index (h+r)*W + w of sample shi*128+p
    idx = nc.dram_tensor("idx", [P, NROW], i32, kind="ExternalInput")
    out = nc.dram_tensor("out", [P, NSHI], f32, kind="ExternalOutput")

    with tile.TileContext(nc) as tc, ExitStack() as ctx:
        sb = ctx.enter_context(tc.tile_pool(name="sb", bufs=1))

        idx_sb = sb.tile([P, NROW], i32)
        nc.sync.dma_start(out=idx_sb[:], in_=idx[:])
        deltas = sb.tile([P, 1], f32)
        nc.vector.memset(deltas[:], DELTA)

        # Per-phase tiles (phase = one 128-sample layer)
        qk = [sb.tile([P, 9, 2 * C], f32, name=f"qk{s}") for s in range(NSHI)]
        d = [sb.tile([P, 9, 2 * C], f32, name=f"d{s}") for s in range(NSHI)]
        d2 = [sb.tile([P, 9, 2 * C], f32, name=f"d2{s}") for s in range(NSHI)]
        n2 = [sb.tile([P, 9, 2, 1], f32, name=f"n2{s}") for s in range(NSHI)]
        rinv = [sb.tile([P, 9, 2, 1], f32, name=f"ri{s}") for s in range(NSHI)]
        dif = [sb.tile([P, 9, C], f32, name=f"df{s}") for s in range(NSHI)]
        acc = sb.tile([P, NSHI], f32)

        # Gathers first: one indirect DMA per phase; per index, 384
        # consecutive floats from fqkT (= 3 w-positions x 128 channels)
        # land contiguously in one partition.
        for s in range(NSHI):
            nc.gpsimd.indirect_dma_start(
                out=qk[s][:].rearrange("p n c -> p (n c)"),
                out_offset=None,
                in_=fqkT[:],
                in_offset=bass.IndirectOffsetOnAxis(
                    ap=idx_sb[:, 3 * s : 3 * s + 3], axis=0
                ),
            )

        def sub(s):  # d = window - center (center column pos=4 becomes 0)
            nc.vector.tensor_tensor(
                out=d[s][:],
                in0=qk[s][:],
                in1=qk[s][:, 4:5, :].to_broadcast([P, 9, 2 * C]),
                op=mybir.AluOpType.subtract,
            )

        def square(s):
            nc.scalar.square(out=d2[s][:], in_=d[s][:])

        def red(s):  # norm2 over each 64-channel run, q and k separately
            nc.vector.tensor_reduce(
                out=n2[s][:],
                in_=d2[s][:].rearrange("p n (t c) -> p n t c", t=2),
                axis=mybir.AxisListType.X,
                op=mybir.AluOpType.add,
            )

        def vsqrt(s):  # nrm = sqrt(n2 + delta)
            nc.scalar.activation(
                out=n2[s][:],
                in_=n2[s][:],
                func=mybir.ActivationFunctionType.Sqrt,
                bias=deltas[:],
            )

        def recip(s):
            nc.vector.reciprocal(out=rinv[s][:], in_=n2[s][:])

        def mult(s):  # qhat/khat = d * rinv; center cols give 0 * (1/1e-7) = 0
            nc.vector.tensor_tensor(
                out=d2[s][:].rearrange("p n (t c) -> p n t c", t=2),
                in0=d[s][:].rearrange("p n (t c) -> p n t c", t=2),
                in1=rinv[s][:].to_broadcast([P, 9, 2, C]),
                op=mybir.AluOpType.mult,
            )

        def qkdif(s):  # on gpsimd, off the DVE critical path
            qhv = d2[s][:].rearrange("p n (t c) -> p n t c", t=2)
            nc.gpsimd.tensor_tensor(
                out=dif[s][:],
                in0=qhv[:, :, 0, :],
                in1=qhv[:, :, 1, :],
                op=mybir.AluOpType.subtract,
            )

        def absred(s):
            nc.vector.tensor_reduce(
                out=acc[:, s : s + 1],
                in_=dif[s][:],
                axis=mybir.AxisListType.XY,
                op=mybir.AluOpType.add,
                apply_absolute_value=True,
            )

        # Two-phase software pipeline; emission order fixes per-engine
        # queue order, Tile inserts the cross-engine semaphores.
        sub(0)
        square(0)
        sub(1)
        red(0)
        square(1)
        vsqrt(0)
        recip(0)
        mult(0)
        qkdif(0)
        red(1)
        vsqrt(1)
        absred(0)
        recip(1)
        mult(1)
        qkdif(1)
        absred(1)
        nc.sync.dma_start(out=out[:], in_=acc[:])

    _split_multi_waits(nc)
    return nc


def kernel(feat_q, feat_k, sample_ids, *, trace=False, trace_cores=None):
    global LAST_RESULTS, _nc_cache
    feat_q = np.asarray(feat_q, dtype=np.float32)
    feat_k = np.asarray(feat_k, dtype=np.float32)
    ids = np.asarray(sample_ids).astype(np.int64)
    if _nc_cache is None:
        _nc_cache = _build()
    nc = _nc_cache

    # idx[p, shi*3 + r] = (h + r)*W + w for sample s = shi*128 + p
    hw = ids[:, 0] * W + ids[:, 1]  # [256]
    rows = hw[:, None] + np.arange(3, dtype=np.int64)[None, :] * W  # [256, 3]
    idx_np = np.ascontiguousarray(
        rows.reshape(NSHI, P, 3).transpose(1, 0, 2).reshape(P, NROW)
    ).astype(np.int32)

    in_maps = []
    for b in range(N_CORES):
        x = np.concatenate([feat_q[b], feat_k[b]], axis=0)  # [128, H, W]
        fqkT = np.ascontiguousarray(x.transpose(1, 2, 0)).reshape(H * W, 2 * C)
        in_maps.append({"fqkT": fqkT, "idx": idx_np})

    results = run_bass_kernel_spmd(
        nc,
        in_maps,
        core_ids=list(range(N_CORES)),
        trace=trace,
        trace_cores=trace_cores,
    )
    LAST_RESULTS = results
    total = np.float64(0.0)
    for r in results.results:
        total += np.float64(np.sum(np.asarray(r["out"], dtype=np.float64)))
    loss = total / (B * C * 8 * NUM_S)
    return np.asarray(loss, dtype=np.float32)
